# revision 10
# baseline (speedup 1.0000x reference)
"""Trainium2 Bass kernel for a minimal Mamba layer (B=2, L=2048, d_model=1024,
d_inner=2048, d_state=16, d_conv=4, dt_rank=64) on 8 NeuronCores.

Sharding: core = (batch, d_inner-quarter).  Cores 0-3 handle batch 0, cores
4-7 batch 1; within a batch group each core owns 512 d_inner channels.

Two SPMD kernels with a tiny host exchange between them:
  A: in_proj (own rows) + causal depthwise conv (as 4 PSUM-accumulated
     diagonal matmuls) + silu + x_proj partial (own-channel contraction).
  host: sum the 4 partial dbc's per batch (96x2048 each), build broadcast
     tiles for B/C rows.
  B: dt_proj + softplus, then per (state, ch-block): dA = exp(A*delta) on
     ScalarE, Bu on VectorE, the SSM recurrence via the hardware
     tensor_tensor_scan, y accumulation, gating, out_proj partial.
  host: sum the 4 partial outputs per batch.
"""

import sys

if "/opt/trn_rl_repo" not in sys.path:
    sys.path.insert(0, "/opt/trn_rl_repo")

import numpy as np
import ml_dtypes

import concourse.bass as bass
from concourse import bacc, mybir
from concourse.bass_utils import run_bass_kernel_spmd
from concourse.tile import TileContext

F32 = mybir.dt.float32
BF16 = mybir.dt.bfloat16
AF = mybir.ActivationFunctionType
OP = mybir.AluOpType

D_MODEL = 1024
D_STATE = 16
D_CONV = 4
D_INNER = 2048
DT_RANK = 64
B = 2
L = 2048
NCORES = 8
CH = D_INNER // 4          # 512 channels per core
NCB = CH // 128            # 4 channel blocks of 128
NT = L // 512              # 4 token tiles of 512
KM = D_MODEL // 128        # 8 k tiles for in_proj

_CACHE = {}


def _build_a():
    nc = bacc.Bacc("TRN2", target_bir_lowering=False, debug=False,
                   num_devices=NCORES)
    xT = nc.dram_tensor("xT", [D_MODEL, L], BF16, kind="ExternalInput").ap()
    w1t = nc.dram_tensor("w1t", [D_MODEL, 2 * CH], BF16, kind="ExternalInput").ap()
    convdiag = nc.dram_tensor("convdiag", [D_CONV * NCB * 128, 128], BF16,
                              kind="ExternalInput").ap()
    convb = nc.dram_tensor("convb", [128, NCB], F32, kind="ExternalInput").ap()
    wxpT = nc.dram_tensor("wxpT", [CH, 96], BF16, kind="ExternalInput").ap()

    xc_out = nc.dram_tensor("xc", [CH, L], BF16, kind="ExternalOutput").ap()
    sres_out = nc.dram_tensor("sres", [CH, L], BF16, kind="ExternalOutput").ap()
    dbc_out = nc.dram_tensor("dbc", [96, L], F32, kind="ExternalOutput").ap()

    with TileContext(nc) as tc:
        with (
            tc.tile_pool(name="const", bufs=1) as const,
            tc.tile_pool(name="psum", bufs=4, space="PSUM") as psum,
            tc.tile_pool(name="work", bufs=3) as work,
        ):
            xT_t = []
            for k in range(KM):
                t = const.tile([128, L], BF16, tag=f"xT{k}")
                nc.sync.dma_start(out=t[:], in_=xT[k * 128:(k + 1) * 128, :])
                xT_t.append(t)
            w1_t = []
            for k in range(KM):
                t = const.tile([128, 2 * CH], BF16, tag=f"w1{k}")
                nc.sync.dma_start(out=t[:], in_=w1t[k * 128:(k + 1) * 128, :])
                w1_t.append(t)
            cdiag = []
            for tap in range(D_CONV):
                row = []
                for cb in range(NCB):
                    t = const.tile([128, 128], BF16, tag=f"cd{tap}_{cb}")
                    off = (tap * NCB + cb) * 128
                    nc.sync.dma_start(out=t[:], in_=convdiag[off:off + 128, :])
                    row.append(t)
                cdiag.append(row)
            cb_t = const.tile([128, NCB], F32, tag="convb")
            nc.sync.dma_start(out=cb_t[:], in_=convb[:])
            wxp_t = []
            for kc in range(NCB):
                t = const.tile([128, 96], BF16, tag=f"wxp{kc}")
                nc.sync.dma_start(out=t[:], in_=wxpT[kc * 128:(kc + 1) * 128, :])
                wxp_t.append(t)

            # xi (post in_proj, pre conv): padded with 3 zero columns in front
            xi_pad = []
            for cb in range(NCB):
                t = const.tile([128, L + D_CONV - 1], BF16, tag=f"xip{cb}")
                nc.vector.memset(t[:, 0:D_CONV - 1], 0.0)
                xi_pad.append(t)
            xc_t = [const.tile([128, L], BF16, tag=f"xc{cb}", name=f"xc{cb}") for cb in range(NCB)]

            # ---- in_proj: out rows 0..511 = xi slice, 512..1023 = res slice
            for n in range(NT):
                for m in range(2 * NCB):
                    pt = psum.tile([128, 512], F32, tag="mm")
                    for k in range(KM):
                        nc.tensor.matmul(
                            pt[:], w1_t[k][:, m * 128:(m + 1) * 128],
                            xT_t[k][:, n * 512:(n + 1) * 512],
                            start=(k == 0), stop=(k == KM - 1))
                    if m < NCB:
                        nc.scalar.activation(
                            xi_pad[m][:, D_CONV - 1 + n * 512:
                                      D_CONV - 1 + (n + 1) * 512],
                            pt[:], AF.Copy)
                    else:
                        st = work.tile([128, 512], BF16, tag="sres")
                        nc.scalar.activation(st[:], pt[:], AF.Silu)
                        nc.sync.dma_start(
                            out=sres_out[(m - NCB) * 128:(m - NCB + 1) * 128,
                                         n * 512:(n + 1) * 512],
                            in_=st[:])

            # ---- causal depthwise conv as 4 accumulated diagonal matmuls
            for cb in range(NCB):
                for n in range(NT):
                    pt = psum.tile([128, 512], F32, tag="mm")
                    for tap in range(D_CONV):
                        nc.tensor.matmul(
                            pt[:], cdiag[tap][cb][:],
                            xi_pad[cb][:, n * 512 + tap:n * 512 + tap + 512],
                            start=(tap == 0), stop=(tap == D_CONV - 1))
                    nc.scalar.activation(
                        xc_t[cb][:, n * 512:(n + 1) * 512], pt[:], AF.Silu,
                        bias=cb_t[:, cb:cb + 1])
                nc.sync.dma_start(out=xc_out[cb * 128:(cb + 1) * 128, :],
                                  in_=xc_t[cb][:])

            # ---- x_proj partial: dbc = wxpT.T @ xc   [96, L]
            for n in range(NT):
                pt = psum.tile([96, 512], F32, tag="xp")
                for kc in range(NCB):
                    nc.tensor.matmul(
                        pt[:], wxp_t[kc][:],
                        xc_t[kc][:, n * 512:(n + 1) * 512],
                        start=(kc == 0), stop=(kc == NCB - 1))
                dt = work.tile([96, 512], F32, tag="dbc")
                nc.scalar.activation(dt[:], pt[:], AF.Copy)
                nc.sync.dma_start(out=dbc_out[:, n * 512:(n + 1) * 512],
                                  in_=dt[:])
    nc.compile()
    return nc


def _build_b():
    nc = bacc.Bacc("TRN2", target_bir_lowering=False, debug=False,
                   num_devices=NCORES)
    xc_in = nc.dram_tensor("xc", [CH, L], BF16, kind="ExternalInput").ap()
    sres_in = nc.dram_tensor("sres", [CH, L], BF16, kind="ExternalInput").ap()
    dtr = nc.dram_tensor("dtr", [DT_RANK, L], F32, kind="ExternalInput").ap()
    brep = nc.dram_tensor("brep", [D_STATE * 128, L], BF16,
                          kind="ExternalInput").ap()
    crep = nc.dram_tensor("crep", [D_STATE * 128, L], BF16,
                          kind="ExternalInput").ap()
    wdtT = nc.dram_tensor("wdtT", [DT_RANK, CH], F32, kind="ExternalInput").ap()
    dtb = nc.dram_tensor("dtb", [128, NCB], F32, kind="ExternalInput").ap()
    woutT = nc.dram_tensor("woutT", [CH, D_MODEL], BF16,
                           kind="ExternalInput").ap()
    acol = nc.dram_tensor("acol", [128, D_STATE * NCB], F32,
                          kind="ExternalInput").ap()
    dcol = nc.dram_tensor("dcol", [128, NCB], F32, kind="ExternalInput").ap()
    ident = nc.dram_tensor("ident", [128, 128], BF16, kind="ExternalInput").ap()

    outp = nc.dram_tensor("outp", [D_MODEL, L], F32, kind="ExternalOutput").ap()

    with TileContext(nc) as tc:
        with (
            tc.tile_pool(name="const", bufs=1) as const,
            tc.tile_pool(name="bc", bufs=5) as bcpool,
            tc.tile_pool(name="work", bufs=3) as work,
        ):
            xc_t, sres_t = [], []
            for cb in range(NCB):
                t = const.tile([128, L], BF16, tag=f"xc{cb}")
                nc.sync.dma_start(out=t[:], in_=xc_in[cb * 128:(cb + 1) * 128, :])
                xc_t.append(t)
                t = const.tile([128, L], BF16, tag=f"sr{cb}")
                nc.sync.dma_start(out=t[:], in_=sres_in[cb * 128:(cb + 1) * 128, :])
                sres_t.append(t)
            dtr_t = const.tile([DT_RANK, L], F32, tag="dtr")
            nc.sync.dma_start(out=dtr_t[:], in_=dtr[:])
            wdt_t = const.tile([DT_RANK, CH], F32, tag="wdt")
            nc.sync.dma_start(out=wdt_t[:], in_=wdtT[:])
            dtb_t = const.tile([128, NCB], F32, tag="dtb")
            nc.sync.dma_start(out=dtb_t[:], in_=dtb[:])
            acol_t = const.tile([128, D_STATE * NCB], F32, tag="acol")
            nc.sync.dma_start(out=acol_t[:], in_=acol[:])
            dcol_t = const.tile([128, NCB], F32, tag="dcol")
            nc.sync.dma_start(out=dcol_t[:], in_=dcol[:])
            id_t = const.tile([128, 128], BF16, tag="ident")
            nc.sync.dma_start(out=id_t[:], in_=ident[:])
            wout_t = []
            for kc in range(NCB):
                t = const.tile([128, D_MODEL], BF16, tag=f"wo{kc}")
                nc.sync.dma_start(out=t[:], in_=woutT[kc * 128:(kc + 1) * 128, :])
                wout_t.append(t)

            # ---- dt_proj + softplus -> delta [CH, L] fp32
            delta_t = []
            with tc.tile_pool(name="psum1", bufs=4, space="PSUM") as psum1:
              for m in range(NCB):
                dt = const.tile([128, L], F32, tag=f"dl{m}")
                ets = []
                for n in range(NT):
                    pt = psum1.tile([128, 512], F32, tag="mm")
                    nc.tensor.matmul(pt[:], wdt_t[:, m * 128:(m + 1) * 128],
                                     dtr_t[:, n * 512:(n + 1) * 512],
                                     start=True, stop=True)
                    # softplus(z) = ln(exp(z) + 1); batching the Exps then the
                    # Lns avoids ACT-table ping-pong
                    et = work.tile([128, 512], F32, tag="spe", bufs=4,
                                   name=f"spe{m}_{n}")
                    nc.scalar.activation(et[:], pt[:], AF.Exp,
                                         bias=dtb_t[:, m:m + 1])
                    ets.append(et)
                for n in range(NT):
                    nc.scalar.activation(dt[:, n * 512:(n + 1) * 512], ets[n][:],
                                         AF.Ln, bias=1.0)
                delta_t.append(dt)
              # (psum1 released before the scan's accumulator pool opens)

            # ---- u = delta * xc (bf16)
            u_t = []
            for cb in range(NCB):
                ut = const.tile([128, L], BF16, tag=f"u{cb}")
                nc.vector.tensor_mul(ut[:], delta_t[cb][:], xc_t[cb][:])
                u_t.append(ut)

            # ---- the scan: per (state, channel-block); the 16 C-weighted
            # state contributions are summed on the PE via identity-matmul
            # accumulation into PSUM (fp32).  Two half-passes of 2 channel
            # blocks each so 2x[128,2048] fp32 accumulators fill all 8 banks.
            y_t = [None] * NCB
            for half in range(2):
                cbs = [2 * half, 2 * half + 1]
                with tc.tile_pool(name=f"accp{half}", bufs=1,
                                  space="PSUM") as accpool:
                    accp = {}
                    for cb in cbs:
                        accp[cb] = accpool.tile([128, L], F32, tag=f"ac{cb}",
                                                name=f"accp{cb}")
                    for s in range(D_STATE):
                        br = bcpool.tile([128, L], BF16, tag="brep")
                        nc.sync.dma_start(out=br[:],
                                          in_=brep[s * 128:(s + 1) * 128, :])
                        cr = bcpool.tile([128, L], BF16, tag="crep")
                        nc.sync.dma_start(out=cr[:],
                                          in_=crep[s * 128:(s + 1) * 128, :])
                        for cb in cbs:
                            dA = work.tile([128, L], BF16, tag="dA")
                            nc.scalar.activation(dA[:], delta_t[cb][:], AF.Exp,
                                                 scale=acol_t[:, s * NCB + cb:
                                                              s * NCB + cb + 1])
                            bu = work.tile([128, L], BF16, tag="bu")
                            nc.vector.tensor_mul(bu[:], u_t[cb][:], br[:])
                            h = work.tile([128, L], BF16, tag="h")
                            nc.vector.tensor_tensor_scan(h[:], dA[:], bu[:], 0.0,
                                                         OP.mult, OP.add)
                            hc = work.tile([128, L], BF16, tag="hc")
                            nc.vector.tensor_mul(hc[:], h[:], cr[:])
                            for n in range(NT):
                                nc.tensor.matmul(
                                    accp[cb][:, n * 512:(n + 1) * 512],
                                    id_t[:],
                                    hc[:, n * 512:(n + 1) * 512],
                                    start=(s == 0), stop=(s == D_STATE - 1))
                    # ---- y = (acc + xc * D) * sres; y overwrites the spent
                    # xc tile (WAR handled by tile dep tracking)
                    for cb in cbs:
                        for n in range(NT):
                            sl = slice(n * 512, (n + 1) * 512)
                            t1 = work.tile([128, 512], BF16, tag="t1")
                            nc.vector.scalar_tensor_tensor(
                                t1[:], xc_t[cb][:, sl], dcol_t[:, cb:cb + 1],
                                accp[cb][:, sl], OP.mult, OP.add)
                            nc.vector.tensor_mul(xc_t[cb][:, sl], t1[:],
                                                 sres_t[cb][:, sl])
                        y_t[cb] = xc_t[cb]

            # ---- out_proj partial: outp = woutT.T @ y  [D_MODEL, L]
            with tc.tile_pool(name="psum2", bufs=4, space="PSUM") as psum2:
              for n in range(NT):
                for m in range(D_MODEL // 128):
                    pt = psum2.tile([128, 512], F32, tag="mm")
                    for kc in range(NCB):
                        nc.tensor.matmul(pt[:],
                                         wout_t[kc][:, m * 128:(m + 1) * 128],
                                         y_t[kc][:, n * 512:(n + 1) * 512],
                                         start=(kc == 0), stop=(kc == NCB - 1))
                    ot = work.tile([128, 512], F32, tag="ot")
                    nc.scalar.activation(ot[:], pt[:], AF.Copy)
                    nc.sync.dma_start(
                        out=outp[m * 128:(m + 1) * 128, n * 512:(n + 1) * 512],
                        in_=ot[:])
              # end psum2
    nc.compile()
    return nc


def _bf(a):
    return np.ascontiguousarray(a).astype(ml_dtypes.bfloat16)


def _f32(a):
    return np.ascontiguousarray(a, dtype=np.float32)


def kernel(x, in_proj_w, conv_w, conv_b, x_proj_w, dt_proj_w, dt_proj_b,
           A_log, D, out_proj_w):
    if "a" not in _CACHE:
        _CACHE["a"] = _build_a()
    if "b" not in _CACHE:
        _CACHE["b"] = _build_b()
    nca, ncb = _CACHE["a"], _CACHE["b"]

    A = -np.exp(np.asarray(A_log, np.float32))          # [D_INNER, D_STATE]
    x = np.asarray(x, np.float32)

    core_bq = [(c // 4, c % 4) for c in range(NCORES)]

    # ---------------- kernel A inputs
    xTb = [_bf(x[b].T) for b in range(B)]
    in_maps = []
    for b, q in core_bq:
        sl = slice(q * CH, (q + 1) * CH)
        w1 = np.concatenate([in_proj_w[sl], in_proj_w[D_INNER + q * CH:
                                                      D_INNER + (q + 1) * CH]], 0)
        cw = conv_w[sl, 0, :]                            # [CH, 4]
        cd = np.zeros((D_CONV * NCB * 128, 128), np.float32)
        for tap in range(D_CONV):
            for cb in range(NCB):
                blk = cd[(tap * NCB + cb) * 128:(tap * NCB + cb + 1) * 128]
                np.fill_diagonal(blk, cw[cb * 128:(cb + 1) * 128, tap])
        in_maps.append({
            "xT": xTb[b],
            "w1t": _bf(w1.T),
            "convdiag": _bf(cd),
            "convb": _f32(conv_b[sl].reshape(NCB, 128).T),
            "wxpT": _bf(x_proj_w[:, sl].T),
        })
    ra = run_bass_kernel_spmd(nca, in_maps, list(range(NCORES)))

    # ---------------- host exchange
    dbc = [None, None]
    for b in range(B):
        dbc[b] = sum(np.asarray(ra.results[4 * b + q]["dbc"], np.float32)
                     for q in range(4))
    in_maps_b = []
    breps, creps = [], []
    for b in range(B):
        Bm = dbc[b][DT_RANK:DT_RANK + D_STATE]           # [16, L]
        Cm = dbc[b][DT_RANK + D_STATE:]
        breps.append(_bf(np.repeat(Bm, 128, axis=0)))
        creps.append(_bf(np.repeat(Cm, 128, axis=0)))
    for c, (b, q) in enumerate(core_bq):
        sl = slice(q * CH, (q + 1) * CH)
        acol = np.zeros((128, D_STATE * NCB), np.float32)
        for s in range(D_STATE):
            for cb in range(NCB):
                acol[:, s * NCB + cb] = A[q * CH + cb * 128:
                                          q * CH + (cb + 1) * 128, s]
        in_maps_b.append({
            "xc": ra.results[c]["xc"],
            "sres": ra.results[c]["sres"],
            "dtr": _f32(dbc[b][:DT_RANK]),
            "brep": breps[b],
            "crep": creps[b],
            "wdtT": _f32(dt_proj_w[sl].T),
            "dtb": _f32(dt_proj_b[sl].reshape(NCB, 128).T),
            "woutT": _bf(out_proj_w[:, sl].T),
            "acol": acol,
            "dcol": _f32(D[sl].reshape(NCB, 128).T),
            "ident": _bf(np.eye(128, dtype=np.float32)),
        })
    rb = run_bass_kernel_spmd(ncb, in_maps_b, list(range(NCORES)))

    out = np.zeros((B, L, D_MODEL), np.float32)
    for b in range(B):
        acc = sum(np.asarray(rb.results[4 * b + q]["outp"], np.float32)
                  for q in range(4))
        out[b] = acc.T
    return out


# revision 11
# speedup vs baseline: 1.0010x; 1.0010x over previous
"""Trainium2 Bass kernel for a minimal Mamba layer (B=2, L=2048, d_model=1024,
d_inner=2048, d_state=16, d_conv=4, dt_rank=64) on 8 NeuronCores.

Sharding: core = (batch, d_inner-quarter).  Cores 0-3 handle batch 0, cores
4-7 batch 1; within a batch group each core owns 512 d_inner channels.

Two SPMD kernels with a tiny host exchange between them:
  A: in_proj (own rows) + causal depthwise conv (as 4 PSUM-accumulated
     diagonal matmuls) + silu + x_proj partial (own-channel contraction).
  host: sum the 4 partial dbc's per batch (96x2048 each), build broadcast
     tiles for B/C rows.
  B: dt_proj + softplus, then per (state, ch-block): dA = exp(A*delta) on
     ScalarE, Bu on VectorE, the SSM recurrence via the hardware
     tensor_tensor_scan, y accumulation, gating, out_proj partial.
  host: sum the 4 partial outputs per batch.
"""

import sys

if "/opt/trn_rl_repo" not in sys.path:
    sys.path.insert(0, "/opt/trn_rl_repo")

import numpy as np
import ml_dtypes

import concourse.bass as bass
from concourse import bacc, mybir
from concourse.bass_utils import run_bass_kernel_spmd
from concourse.tile import TileContext

F32 = mybir.dt.float32
BF16 = mybir.dt.bfloat16
AF = mybir.ActivationFunctionType
OP = mybir.AluOpType

D_MODEL = 1024
D_STATE = 16
D_CONV = 4
D_INNER = 2048
DT_RANK = 64
B = 2
L = 2048
NCORES = 8
CH = D_INNER // 4          # 512 channels per core
NCB = CH // 128            # 4 channel blocks of 128
NT = L // 512              # 4 token tiles of 512
KM = D_MODEL // 128        # 8 k tiles for in_proj

_CACHE = {}


def _build_a():
    nc = bacc.Bacc("TRN2", target_bir_lowering=False, debug=False,
                   num_devices=NCORES)
    xT = nc.dram_tensor("xT", [D_MODEL, L], BF16, kind="ExternalInput").ap()
    w1t = nc.dram_tensor("w1t", [D_MODEL, 2 * CH], BF16, kind="ExternalInput").ap()
    convdiag = nc.dram_tensor("convdiag", [D_CONV * NCB * 128, 128], BF16,
                              kind="ExternalInput").ap()
    convb = nc.dram_tensor("convb", [128, NCB], F32, kind="ExternalInput").ap()
    wxpT = nc.dram_tensor("wxpT", [CH, 96], BF16, kind="ExternalInput").ap()

    xc_out = nc.dram_tensor("xc", [CH, L], BF16, kind="ExternalOutput").ap()
    sres_out = nc.dram_tensor("sres", [CH, L], BF16, kind="ExternalOutput").ap()
    dbc_out = nc.dram_tensor("dbc", [96, L], F32, kind="ExternalOutput").ap()

    with TileContext(nc) as tc:
        with (
            tc.tile_pool(name="const", bufs=1) as const,
            tc.tile_pool(name="psum", bufs=4, space="PSUM") as psum,
            tc.tile_pool(name="work", bufs=3) as work,
        ):
            xT_t = []
            for k in range(KM):
                t = const.tile([128, L], BF16, tag=f"xT{k}")
                nc.sync.dma_start(out=t[:], in_=xT[k * 128:(k + 1) * 128, :])
                xT_t.append(t)
            w1_t = []
            for k in range(KM):
                t = const.tile([128, 2 * CH], BF16, tag=f"w1{k}")
                nc.sync.dma_start(out=t[:], in_=w1t[k * 128:(k + 1) * 128, :])
                w1_t.append(t)
            cdiag = []
            for tap in range(D_CONV):
                row = []
                for cb in range(NCB):
                    t = const.tile([128, 128], BF16, tag=f"cd{tap}_{cb}")
                    off = (tap * NCB + cb) * 128
                    nc.sync.dma_start(out=t[:], in_=convdiag[off:off + 128, :])
                    row.append(t)
                cdiag.append(row)
            cb_t = const.tile([128, NCB], F32, tag="convb")
            nc.sync.dma_start(out=cb_t[:], in_=convb[:])
            wxp_t = []
            for kc in range(NCB):
                t = const.tile([128, 96], BF16, tag=f"wxp{kc}")
                nc.sync.dma_start(out=t[:], in_=wxpT[kc * 128:(kc + 1) * 128, :])
                wxp_t.append(t)

            # xi (post in_proj, pre conv): padded with 3 zero columns in front
            xi_pad = []
            for cb in range(NCB):
                t = const.tile([128, L + D_CONV - 1], BF16, tag=f"xip{cb}")
                nc.vector.memset(t[:, 0:D_CONV - 1], 0.0)
                xi_pad.append(t)
            xc_t = [const.tile([128, L], BF16, tag=f"xc{cb}", name=f"xc{cb}") for cb in range(NCB)]

            # ---- in_proj: out rows 0..511 = xi slice, 512..1023 = res slice
            for n in range(NT):
                for m in range(2 * NCB):
                    pt = psum.tile([128, 512], F32, tag="mm")
                    for k in range(KM):
                        nc.tensor.matmul(
                            pt[:], w1_t[k][:, m * 128:(m + 1) * 128],
                            xT_t[k][:, n * 512:(n + 1) * 512],
                            start=(k == 0), stop=(k == KM - 1))
                    if m < NCB:
                        nc.scalar.activation(
                            xi_pad[m][:, D_CONV - 1 + n * 512:
                                      D_CONV - 1 + (n + 1) * 512],
                            pt[:], AF.Copy)
                    else:
                        st = work.tile([128, 512], BF16, tag="sres")
                        nc.scalar.activation(st[:], pt[:], AF.Silu)
                        nc.sync.dma_start(
                            out=sres_out[(m - NCB) * 128:(m - NCB + 1) * 128,
                                         n * 512:(n + 1) * 512],
                            in_=st[:])

            # ---- causal depthwise conv as 4 accumulated diagonal matmuls
            for cb in range(NCB):
                for n in range(NT):
                    pt = psum.tile([128, 512], F32, tag="mm")
                    for tap in range(D_CONV):
                        nc.tensor.matmul(
                            pt[:], cdiag[tap][cb][:],
                            xi_pad[cb][:, n * 512 + tap:n * 512 + tap + 512],
                            start=(tap == 0), stop=(tap == D_CONV - 1))
                    nc.scalar.activation(
                        xc_t[cb][:, n * 512:(n + 1) * 512], pt[:], AF.Silu,
                        bias=cb_t[:, cb:cb + 1])
                nc.sync.dma_start(out=xc_out[cb * 128:(cb + 1) * 128, :],
                                  in_=xc_t[cb][:])

            # ---- x_proj partial: dbc = wxpT.T @ xc   [96, L]
            for n in range(NT):
                pt = psum.tile([96, 512], F32, tag="xp")
                for kc in range(NCB):
                    nc.tensor.matmul(
                        pt[:], wxp_t[kc][:],
                        xc_t[kc][:, n * 512:(n + 1) * 512],
                        start=(kc == 0), stop=(kc == NCB - 1))
                dt = work.tile([96, 512], F32, tag="dbc")
                nc.scalar.activation(dt[:], pt[:], AF.Copy)
                nc.sync.dma_start(out=dbc_out[:, n * 512:(n + 1) * 512],
                                  in_=dt[:])
    nc.compile()
    return nc


def _build_b():
    nc = bacc.Bacc("TRN2", target_bir_lowering=False, debug=False,
                   num_devices=NCORES)
    xc_in = nc.dram_tensor("xc", [CH, L], BF16, kind="ExternalInput").ap()
    sres_in = nc.dram_tensor("sres", [CH, L], BF16, kind="ExternalInput").ap()
    dtr = nc.dram_tensor("dtr", [DT_RANK, L], F32, kind="ExternalInput").ap()
    brep = nc.dram_tensor("brep", [D_STATE * 128, L], BF16,
                          kind="ExternalInput").ap()
    crep = nc.dram_tensor("crep", [D_STATE * 128, L], BF16,
                          kind="ExternalInput").ap()
    wdtT = nc.dram_tensor("wdtT", [DT_RANK, CH], F32, kind="ExternalInput").ap()
    dtb = nc.dram_tensor("dtb", [128, NCB], F32, kind="ExternalInput").ap()
    woutT = nc.dram_tensor("woutT", [CH, D_MODEL], BF16,
                           kind="ExternalInput").ap()
    acol = nc.dram_tensor("acol", [128, D_STATE * NCB], F32,
                          kind="ExternalInput").ap()
    dcol = nc.dram_tensor("dcol", [128, NCB], F32, kind="ExternalInput").ap()
    ident = nc.dram_tensor("ident", [128, 128], BF16, kind="ExternalInput").ap()

    outp = nc.dram_tensor("outp", [D_MODEL, L], F32, kind="ExternalOutput").ap()

    with TileContext(nc) as tc:
        with (
            tc.tile_pool(name="const", bufs=1) as const,
            tc.tile_pool(name="bc", bufs=4) as bcpool,
            tc.tile_pool(name="dap", bufs=4) as dapool,
            tc.tile_pool(name="work", bufs=3) as work,
        ):
            xc_t, sres_t = [], []
            for cb in range(NCB):
                t = const.tile([128, L], BF16, tag=f"xc{cb}")
                nc.sync.dma_start(out=t[:], in_=xc_in[cb * 128:(cb + 1) * 128, :])
                xc_t.append(t)
                t = const.tile([128, L], BF16, tag=f"sr{cb}")
                nc.sync.dma_start(out=t[:], in_=sres_in[cb * 128:(cb + 1) * 128, :])
                sres_t.append(t)
            dtr_t = const.tile([DT_RANK, L], F32, tag="dtr")
            nc.sync.dma_start(out=dtr_t[:], in_=dtr[:])
            wdt_t = const.tile([DT_RANK, CH], F32, tag="wdt")
            nc.sync.dma_start(out=wdt_t[:], in_=wdtT[:])
            dtb_t = const.tile([128, NCB], F32, tag="dtb")
            nc.sync.dma_start(out=dtb_t[:], in_=dtb[:])
            acol_t = const.tile([128, D_STATE * NCB], F32, tag="acol")
            nc.sync.dma_start(out=acol_t[:], in_=acol[:])
            dcol_t = const.tile([128, NCB], F32, tag="dcol")
            nc.sync.dma_start(out=dcol_t[:], in_=dcol[:])
            id_t = const.tile([128, 128], BF16, tag="ident")
            nc.sync.dma_start(out=id_t[:], in_=ident[:])
            wout_t = []
            for kc in range(NCB):
                t = const.tile([128, D_MODEL], BF16, tag=f"wo{kc}")
                nc.sync.dma_start(out=t[:], in_=woutT[kc * 128:(kc + 1) * 128, :])
                wout_t.append(t)

            # ---- dt_proj + softplus -> delta [CH, L] fp32
            delta_t = []
            with tc.tile_pool(name="psum1", bufs=4, space="PSUM") as psum1:
              for m in range(NCB):
                dt = const.tile([128, L], F32, tag=f"dl{m}")
                ets = []
                for n in range(NT):
                    pt = psum1.tile([128, 512], F32, tag="mm")
                    nc.tensor.matmul(pt[:], wdt_t[:, m * 128:(m + 1) * 128],
                                     dtr_t[:, n * 512:(n + 1) * 512],
                                     start=True, stop=True)
                    # softplus(z) = ln(exp(z) + 1); batching the Exps then the
                    # Lns avoids ACT-table ping-pong
                    et = work.tile([128, 512], F32, tag="spe", bufs=4,
                                   name=f"spe{m}_{n}")
                    nc.scalar.activation(et[:], pt[:], AF.Exp,
                                         bias=dtb_t[:, m:m + 1])
                    ets.append(et)
                for n in range(NT):
                    nc.scalar.activation(dt[:, n * 512:(n + 1) * 512], ets[n][:],
                                         AF.Ln, bias=1.0)
                delta_t.append(dt)
              # (psum1 released before the scan's accumulator pool opens)

            # ---- u = delta * xc (bf16)
            u_t = []
            for cb in range(NCB):
                ut = const.tile([128, L], BF16, tag=f"u{cb}")
                nc.vector.tensor_mul(ut[:], delta_t[cb][:], xc_t[cb][:])
                u_t.append(ut)

            # ---- the scan: per (state, channel-block); the 16 C-weighted
            # state contributions are summed on the PE via identity-matmul
            # accumulation into PSUM (fp32).  Two half-passes of 2 channel
            # blocks each so 2x[128,2048] fp32 accumulators fill all 8 banks.
            y_t = [None] * NCB
            for half in range(2):
                cbs = [2 * half, 2 * half + 1]
                with tc.tile_pool(name=f"accp{half}", bufs=1,
                                  space="PSUM") as accpool:
                    accp = {}
                    for cb in cbs:
                        accp[cb] = accpool.tile([128, L], F32, tag=f"ac{cb}",
                                                name=f"accp{cb}")
                    for s in range(D_STATE):
                        br = bcpool.tile([128, L], BF16, tag="brep")
                        nc.sync.dma_start(out=br[:],
                                          in_=brep[s * 128:(s + 1) * 128, :])
                        cr = bcpool.tile([128, L], BF16, tag="crep")
                        nc.sync.dma_start(out=cr[:],
                                          in_=crep[s * 128:(s + 1) * 128, :])
                        for cb in cbs:
                            dA = dapool.tile([128, L], BF16, tag="dA")
                            nc.scalar.activation(dA[:], delta_t[cb][:], AF.Exp,
                                                 scale=acol_t[:, s * NCB + cb:
                                                              s * NCB + cb + 1])
                            bu = work.tile([128, L], BF16, tag="bu")
                            nc.vector.tensor_mul(bu[:], u_t[cb][:], br[:])
                            h = work.tile([128, L], BF16, tag="h")
                            nc.vector.tensor_tensor_scan(h[:], dA[:], bu[:], 0.0,
                                                         OP.mult, OP.add)
                            hc = work.tile([128, L], BF16, tag="hc")
                            nc.vector.tensor_mul(hc[:], h[:], cr[:])
                            for n in range(NT):
                                nc.tensor.matmul(
                                    accp[cb][:, n * 512:(n + 1) * 512],
                                    id_t[:],
                                    hc[:, n * 512:(n + 1) * 512],
                                    start=(s == 0), stop=(s == D_STATE - 1))
                    # ---- y = (acc + xc * D) * sres; y overwrites the spent
                    # xc tile (WAR handled by tile dep tracking)
                    for cb in cbs:
                        for n in range(NT):
                            sl = slice(n * 512, (n + 1) * 512)
                            t1 = work.tile([128, 512], BF16, tag="t1")
                            nc.vector.scalar_tensor_tensor(
                                t1[:], xc_t[cb][:, sl], dcol_t[:, cb:cb + 1],
                                accp[cb][:, sl], OP.mult, OP.add)
                            nc.vector.tensor_mul(xc_t[cb][:, sl], t1[:],
                                                 sres_t[cb][:, sl])
                        y_t[cb] = xc_t[cb]

            # ---- out_proj partial: outp = woutT.T @ y  [D_MODEL, L]
            with tc.tile_pool(name="psum2", bufs=4, space="PSUM") as psum2:
              for n in range(NT):
                for m in range(D_MODEL // 128):
                    pt = psum2.tile([128, 512], F32, tag="mm")
                    for kc in range(NCB):
                        nc.tensor.matmul(pt[:],
                                         wout_t[kc][:, m * 128:(m + 1) * 128],
                                         y_t[kc][:, n * 512:(n + 1) * 512],
                                         start=(kc == 0), stop=(kc == NCB - 1))
                    ot = work.tile([128, 512], F32, tag="ot")
                    nc.scalar.activation(ot[:], pt[:], AF.Copy)
                    nc.sync.dma_start(
                        out=outp[m * 128:(m + 1) * 128, n * 512:(n + 1) * 512],
                        in_=ot[:])
              # end psum2
    nc.compile()
    return nc


def _bf(a):
    return np.ascontiguousarray(a).astype(ml_dtypes.bfloat16)


def _f32(a):
    return np.ascontiguousarray(a, dtype=np.float32)


def kernel(x, in_proj_w, conv_w, conv_b, x_proj_w, dt_proj_w, dt_proj_b,
           A_log, D, out_proj_w):
    if "a" not in _CACHE:
        _CACHE["a"] = _build_a()
    if "b" not in _CACHE:
        _CACHE["b"] = _build_b()
    nca, ncb = _CACHE["a"], _CACHE["b"]

    A = -np.exp(np.asarray(A_log, np.float32))          # [D_INNER, D_STATE]
    x = np.asarray(x, np.float32)

    core_bq = [(c // 4, c % 4) for c in range(NCORES)]

    # ---------------- kernel A inputs
    xTb = [_bf(x[b].T) for b in range(B)]
    in_maps = []
    for b, q in core_bq:
        sl = slice(q * CH, (q + 1) * CH)
        w1 = np.concatenate([in_proj_w[sl], in_proj_w[D_INNER + q * CH:
                                                      D_INNER + (q + 1) * CH]], 0)
        cw = conv_w[sl, 0, :]                            # [CH, 4]
        cd = np.zeros((D_CONV * NCB * 128, 128), np.float32)
        for tap in range(D_CONV):
            for cb in range(NCB):
                blk = cd[(tap * NCB + cb) * 128:(tap * NCB + cb + 1) * 128]
                np.fill_diagonal(blk, cw[cb * 128:(cb + 1) * 128, tap])
        in_maps.append({
            "xT": xTb[b],
            "w1t": _bf(w1.T),
            "convdiag": _bf(cd),
            "convb": _f32(conv_b[sl].reshape(NCB, 128).T),
            "wxpT": _bf(x_proj_w[:, sl].T),
        })
    ra = run_bass_kernel_spmd(nca, in_maps, list(range(NCORES)))

    # ---------------- host exchange
    dbc = [None, None]
    for b in range(B):
        dbc[b] = sum(np.asarray(ra.results[4 * b + q]["dbc"], np.float32)
                     for q in range(4))
    in_maps_b = []
    breps, creps = [], []
    for b in range(B):
        Bm = dbc[b][DT_RANK:DT_RANK + D_STATE]           # [16, L]
        Cm = dbc[b][DT_RANK + D_STATE:]
        breps.append(_bf(np.repeat(Bm, 128, axis=0)))
        creps.append(_bf(np.repeat(Cm, 128, axis=0)))
    for c, (b, q) in enumerate(core_bq):
        sl = slice(q * CH, (q + 1) * CH)
        acol = np.zeros((128, D_STATE * NCB), np.float32)
        for s in range(D_STATE):
            for cb in range(NCB):
                acol[:, s * NCB + cb] = A[q * CH + cb * 128:
                                          q * CH + (cb + 1) * 128, s]
        in_maps_b.append({
            "xc": ra.results[c]["xc"],
            "sres": ra.results[c]["sres"],
            "dtr": _f32(dbc[b][:DT_RANK]),
            "brep": breps[b],
            "crep": creps[b],
            "wdtT": _f32(dt_proj_w[sl].T),
            "dtb": _f32(dt_proj_b[sl].reshape(NCB, 128).T),
            "woutT": _bf(out_proj_w[:, sl].T),
            "acol": acol,
            "dcol": _f32(D[sl].reshape(NCB, 128).T),
            "ident": _bf(np.eye(128, dtype=np.float32)),
        })
    rb = run_bass_kernel_spmd(ncb, in_maps_b, list(range(NCORES)))

    out = np.zeros((B, L, D_MODEL), np.float32)
    for b in range(B):
        acc = sum(np.asarray(rb.results[4 * b + q]["outp"], np.float32)
                  for q in range(4))
        out[b] = acc.T
    return out


# revision 12
# speedup vs baseline: 1.0307x; 1.0296x over previous
"""Trainium2 Bass kernel for a minimal Mamba layer (B=2, L=2048, d_model=1024,
d_inner=2048, d_state=16, d_conv=4, dt_rank=64) on 8 NeuronCores.

Sharding: core = (batch, d_inner-quarter).  Cores 0-3 handle batch 0, cores
4-7 batch 1; within a batch group each core owns 512 d_inner channels.

Two SPMD kernels with a tiny host exchange between them:
  A: in_proj (own rows) + causal depthwise conv (as 4 PSUM-accumulated
     diagonal matmuls) + silu + x_proj partial (own-channel contraction).
  host: sum the 4 partial dbc's per batch (96x2048 each), build broadcast
     tiles for B/C rows.
  B: dt_proj + softplus, then per (state, ch-block): dA = exp(A*delta) on
     ScalarE, Bu on VectorE, the SSM recurrence via the hardware
     tensor_tensor_scan, y accumulation, gating, out_proj partial.
  host: sum the 4 partial outputs per batch.
"""

import sys

if "/opt/trn_rl_repo" not in sys.path:
    sys.path.insert(0, "/opt/trn_rl_repo")

import numpy as np
import ml_dtypes

import concourse.bass as bass
from concourse import bacc, mybir
from concourse.bass_utils import run_bass_kernel_spmd
from concourse.tile import TileContext

F32 = mybir.dt.float32
BF16 = mybir.dt.bfloat16
AF = mybir.ActivationFunctionType
OP = mybir.AluOpType

D_MODEL = 1024
D_STATE = 16
D_CONV = 4
D_INNER = 2048
DT_RANK = 64
B = 2
L = 2048
NCORES = 8
CH = D_INNER // 4          # 512 channels per core
NCB = CH // 128            # 4 channel blocks of 128
NT = L // 512              # 4 token tiles of 512
KM = D_MODEL // 128        # 8 k tiles for in_proj

_CACHE = {}


def _build_a():
    nc = bacc.Bacc("TRN2", target_bir_lowering=False, debug=False,
                   num_devices=NCORES)
    xT = nc.dram_tensor("xT", [D_MODEL, L], BF16, kind="ExternalInput").ap()
    w1t = nc.dram_tensor("w1t", [D_MODEL, 2 * CH], BF16, kind="ExternalInput").ap()
    convdiag = nc.dram_tensor("convdiag", [D_CONV * NCB * 128, 128], BF16,
                              kind="ExternalInput").ap()
    convb = nc.dram_tensor("convb", [128, NCB], F32, kind="ExternalInput").ap()
    wxpT = nc.dram_tensor("wxpT", [CH, 96], BF16, kind="ExternalInput").ap()

    xc_out = nc.dram_tensor("xc", [CH, L], BF16, kind="ExternalOutput").ap()
    sres_out = nc.dram_tensor("sres", [CH, L], BF16, kind="ExternalOutput").ap()
    dbc_out = nc.dram_tensor("dbc", [96, L], F32, kind="ExternalOutput").ap()

    with TileContext(nc) as tc:
        with (
            tc.tile_pool(name="const", bufs=1) as const,
            tc.tile_pool(name="psum", bufs=4, space="PSUM") as psum,
            tc.tile_pool(name="work", bufs=3) as work,
        ):
            xT_t, w1_t = [], []
            for k in range(KM):
                t = const.tile([128, L], BF16, tag=f"xT{k}")
                nc.sync.dma_start(out=t[:], in_=xT[k * 128:(k + 1) * 128, :])
                xT_t.append(t)
                t = const.tile([128, 2 * CH], BF16, tag=f"w1{k}", name=f"w1{k}")
                nc.sync.dma_start(out=t[:], in_=w1t[k * 128:(k + 1) * 128, :])
                w1_t.append(t)
            cdiag = []
            for tap in range(D_CONV):
                row = []
                for cb in range(NCB):
                    t = const.tile([128, 128], BF16, tag=f"cd{tap}_{cb}")
                    off = (tap * NCB + cb) * 128
                    nc.sync.dma_start(out=t[:], in_=convdiag[off:off + 128, :])
                    row.append(t)
                cdiag.append(row)
            cb_t = const.tile([128, NCB], F32, tag="convb")
            nc.sync.dma_start(out=cb_t[:], in_=convb[:])
            wxp_t = []
            for kc in range(NCB):
                t = const.tile([128, 96], BF16, tag=f"wxp{kc}")
                nc.sync.dma_start(out=t[:], in_=wxpT[kc * 128:(kc + 1) * 128, :])
                wxp_t.append(t)

            # xi (post in_proj, pre conv): padded with 3 zero columns in front
            xi_pad = []
            for cb in range(NCB):
                t = const.tile([128, L + D_CONV - 1], BF16, tag=f"xip{cb}")
                nc.vector.memset(t[:, 0:D_CONV - 1], 0.0)
                xi_pad.append(t)
            xc_t = [const.tile([128, L], BF16, tag=f"xc{cb}", name=f"xc{cb}") for cb in range(NCB)]

            # ---- in_proj: out rows 0..511 = xi slice, 512..1023 = res slice
            for n in range(NT):
                for m in range(2 * NCB):
                    pt = psum.tile([128, 512], F32, tag="mm")
                    for k in range(KM):
                        nc.tensor.matmul(
                            pt[:], w1_t[k][:, m * 128:(m + 1) * 128],
                            xT_t[k][:, n * 512:(n + 1) * 512],
                            start=(k == 0), stop=(k == KM - 1))
                    if m < NCB:
                        nc.scalar.activation(
                            xi_pad[m][:, D_CONV - 1 + n * 512:
                                      D_CONV - 1 + (n + 1) * 512],
                            pt[:], AF.Copy)
                    else:
                        st = work.tile([128, 512], BF16, tag="sres")
                        nc.scalar.activation(st[:], pt[:], AF.Silu)
                        nc.sync.dma_start(
                            out=sres_out[(m - NCB) * 128:(m - NCB + 1) * 128,
                                         n * 512:(n + 1) * 512],
                            in_=st[:])

            # ---- causal depthwise conv as 4 accumulated diagonal matmuls
            for cb in range(NCB):
                for n in range(NT):
                    pt = psum.tile([128, 512], F32, tag="mm")
                    for tap in range(D_CONV):
                        nc.tensor.matmul(
                            pt[:], cdiag[tap][cb][:],
                            xi_pad[cb][:, n * 512 + tap:n * 512 + tap + 512],
                            start=(tap == 0), stop=(tap == D_CONV - 1))
                    nc.scalar.activation(
                        xc_t[cb][:, n * 512:(n + 1) * 512], pt[:], AF.Silu,
                        bias=cb_t[:, cb:cb + 1])
                nc.sync.dma_start(out=xc_out[cb * 128:(cb + 1) * 128, :],
                                  in_=xc_t[cb][:])

            # ---- x_proj partial: dbc = wxpT.T @ xc   [96, L]
            for n in range(NT):
                pt = psum.tile([96, 512], F32, tag="xp")
                for kc in range(NCB):
                    nc.tensor.matmul(
                        pt[:], wxp_t[kc][:],
                        xc_t[kc][:, n * 512:(n + 1) * 512],
                        start=(kc == 0), stop=(kc == NCB - 1))
                dt = work.tile([96, 512], F32, tag="dbc")
                nc.scalar.activation(dt[:], pt[:], AF.Copy)
                nc.sync.dma_start(out=dbc_out[:, n * 512:(n + 1) * 512],
                                  in_=dt[:])
    nc.compile()
    return nc


def _build_b():
    nc = bacc.Bacc("TRN2", target_bir_lowering=False, debug=False,
                   num_devices=NCORES)
    xc_in = nc.dram_tensor("xc", [CH, L], BF16, kind="ExternalInput").ap()
    sres_in = nc.dram_tensor("sres", [CH, L], BF16, kind="ExternalInput").ap()
    dtr = nc.dram_tensor("dtr", [DT_RANK, L], F32, kind="ExternalInput").ap()
    brep = nc.dram_tensor("brep", [D_STATE * 128, L], BF16,
                          kind="ExternalInput").ap()
    crep = nc.dram_tensor("crep", [D_STATE * 128, L], BF16,
                          kind="ExternalInput").ap()
    wdtT = nc.dram_tensor("wdtT", [DT_RANK, CH], F32, kind="ExternalInput").ap()
    dtb = nc.dram_tensor("dtb", [128, NCB], F32, kind="ExternalInput").ap()
    woutT = nc.dram_tensor("woutT", [CH, D_MODEL], BF16,
                           kind="ExternalInput").ap()
    acol = nc.dram_tensor("acol", [128, D_STATE * NCB], F32,
                          kind="ExternalInput").ap()
    dcol = nc.dram_tensor("dcol", [128, NCB], F32, kind="ExternalInput").ap()
    ident = nc.dram_tensor("ident", [128, 128], BF16, kind="ExternalInput").ap()

    outp = nc.dram_tensor("outp", [D_MODEL, L], F32, kind="ExternalOutput").ap()

    with TileContext(nc) as tc:
        with (
            tc.tile_pool(name="const", bufs=1) as const,
            tc.tile_pool(name="bc", bufs=4) as bcpool,
            tc.tile_pool(name="dap", bufs=4) as dapool,
            tc.tile_pool(name="work", bufs=3) as work,
        ):
            dtr_t = const.tile([DT_RANK, L], F32, tag="dtr")
            nc.sync.dma_start(out=dtr_t[:], in_=dtr[:])
            wdt_t = const.tile([DT_RANK, CH], F32, tag="wdt")
            nc.sync.dma_start(out=wdt_t[:], in_=wdtT[:])
            dtb_t = const.tile([128, NCB], F32, tag="dtb")
            nc.sync.dma_start(out=dtb_t[:], in_=dtb[:])
            acol_t = const.tile([128, D_STATE * NCB], F32, tag="acol")
            nc.sync.dma_start(out=acol_t[:], in_=acol[:])
            dcol_t = const.tile([128, NCB], F32, tag="dcol")
            nc.sync.dma_start(out=dcol_t[:], in_=dcol[:])
            id_t = const.tile([128, 128], BF16, tag="ident")
            nc.sync.dma_start(out=id_t[:], in_=ident[:])
            xc_t, sres_t = [], []
            for cb in range(NCB):
                t = const.tile([128, L], BF16, tag=f"xc{cb}")
                nc.sync.dma_start(out=t[:], in_=xc_in[cb * 128:(cb + 1) * 128, :])
                xc_t.append(t)
                t = const.tile([128, L], BF16, tag=f"sr{cb}")
                nc.sync.dma_start(out=t[:], in_=sres_in[cb * 128:(cb + 1) * 128, :])
                sres_t.append(t)
            wout_t = []
            for kc in range(NCB):
                t = const.tile([128, D_MODEL], BF16, tag=f"wo{kc}")
                nc.sync.dma_start(out=t[:], in_=woutT[kc * 128:(kc + 1) * 128, :])
                wout_t.append(t)

            # ---- dt_proj + softplus -> delta [CH, L] fp32
            delta_t = []
            with tc.tile_pool(name="psum1", bufs=4, space="PSUM") as psum1:
              for m in range(NCB):
                dt = const.tile([128, L], F32, tag=f"dl{m}")
                ets = []
                for n in range(NT):
                    pt = psum1.tile([128, 512], F32, tag="mm")
                    nc.tensor.matmul(pt[:], wdt_t[:, m * 128:(m + 1) * 128],
                                     dtr_t[:, n * 512:(n + 1) * 512],
                                     start=True, stop=True)
                    # softplus(z) = ln(exp(z) + 1); batching the Exps then the
                    # Lns avoids ACT-table ping-pong
                    et = work.tile([128, 512], F32, tag="spe", bufs=4,
                                   name=f"spe{m}_{n}")
                    nc.scalar.activation(et[:], pt[:], AF.Exp,
                                         bias=dtb_t[:, m:m + 1])
                    ets.append(et)
                for n in range(NT):
                    nc.scalar.activation(dt[:, n * 512:(n + 1) * 512], ets[n][:],
                                         AF.Ln, bias=1.0)
                delta_t.append(dt)
              # (psum1 released before the scan's accumulator pool opens)

            # ---- u = delta * xc (bf16)
            u_t = []
            for cb in range(NCB):
                ut = const.tile([128, L], BF16, tag=f"u{cb}")
                nc.vector.tensor_mul(ut[:], delta_t[cb][:], xc_t[cb][:])
                u_t.append(ut)

            # ---- the scan: per (state, channel-block); the 16 C-weighted
            # state contributions are summed on the PE via identity-matmul
            # accumulation into PSUM (fp32).  Two half-passes of 2 channel
            # blocks each so 2x[128,2048] fp32 accumulators fill all 8 banks.
            y_t = [None] * NCB
            for half in range(2):
                cbs = [2 * half, 2 * half + 1]
                with tc.tile_pool(name=f"accp{half}", bufs=1,
                                  space="PSUM") as accpool:
                    accp = {}
                    for cb in cbs:
                        accp[cb] = accpool.tile([128, L], F32, tag=f"ac{cb}",
                                                name=f"accp{cb}")
                    for s in range(D_STATE):
                        br = bcpool.tile([128, L], BF16, tag="brep")
                        nc.sync.dma_start(out=br[:],
                                          in_=brep[s * 128:(s + 1) * 128, :])
                        cr = bcpool.tile([128, L], BF16, tag="crep")
                        nc.sync.dma_start(out=cr[:],
                                          in_=crep[s * 128:(s + 1) * 128, :])
                        for cb in cbs:
                            dA = dapool.tile([128, L], BF16, tag="dA")
                            nc.scalar.activation(dA[:], delta_t[cb][:], AF.Exp,
                                                 scale=acol_t[:, s * NCB + cb:
                                                              s * NCB + cb + 1])
                            bu = work.tile([128, L], BF16, tag="bu")
                            nc.vector.tensor_mul(bu[:], u_t[cb][:], br[:])
                            h = work.tile([128, L], BF16, tag="h")
                            nc.vector.tensor_tensor_scan(h[:], dA[:], bu[:], 0.0,
                                                         OP.mult, OP.add)
                            hc = work.tile([128, L], BF16, tag="hc")
                            nc.vector.tensor_mul(hc[:], h[:], cr[:])
                            for n in range(NT):
                                nc.tensor.matmul(
                                    accp[cb][:, n * 512:(n + 1) * 512],
                                    id_t[:],
                                    hc[:, n * 512:(n + 1) * 512],
                                    start=(s == 0), stop=(s == D_STATE - 1))
                    # ---- y = (acc + xc * D) * sres; y overwrites the spent
                    # xc tile (WAR handled by tile dep tracking)
                    for cb in cbs:
                        for n in range(NT):
                            sl = slice(n * 512, (n + 1) * 512)
                            t1 = work.tile([128, 512], BF16, tag="t1")
                            nc.vector.scalar_tensor_tensor(
                                t1[:], xc_t[cb][:, sl], dcol_t[:, cb:cb + 1],
                                accp[cb][:, sl], OP.mult, OP.add)
                            nc.vector.tensor_mul(xc_t[cb][:, sl], t1[:],
                                                 sres_t[cb][:, sl])
                        y_t[cb] = xc_t[cb]

            # ---- out_proj partial: outp = woutT.T @ y  [D_MODEL, L]
            with tc.tile_pool(name="psum2", bufs=4, space="PSUM") as psum2:
              for n in range(NT):
                for m in range(D_MODEL // 128):
                    pt = psum2.tile([128, 512], F32, tag="mm")
                    for kc in range(NCB):
                        nc.tensor.matmul(pt[:],
                                         wout_t[kc][:, m * 128:(m + 1) * 128],
                                         y_t[kc][:, n * 512:(n + 1) * 512],
                                         start=(kc == 0), stop=(kc == NCB - 1))
                    ot = work.tile([128, 512], F32, tag="ot")
                    nc.scalar.activation(ot[:], pt[:], AF.Copy)
                    nc.sync.dma_start(
                        out=outp[m * 128:(m + 1) * 128, n * 512:(n + 1) * 512],
                        in_=ot[:])
              # end psum2
    nc.compile()
    return nc


def _bf(a):
    return np.ascontiguousarray(a).astype(ml_dtypes.bfloat16)


def _f32(a):
    return np.ascontiguousarray(a, dtype=np.float32)


def kernel(x, in_proj_w, conv_w, conv_b, x_proj_w, dt_proj_w, dt_proj_b,
           A_log, D, out_proj_w):
    if "a" not in _CACHE:
        _CACHE["a"] = _build_a()
    if "b" not in _CACHE:
        _CACHE["b"] = _build_b()
    nca, ncb = _CACHE["a"], _CACHE["b"]

    A = -np.exp(np.asarray(A_log, np.float32))          # [D_INNER, D_STATE]
    x = np.asarray(x, np.float32)

    core_bq = [(c // 4, c % 4) for c in range(NCORES)]

    # ---------------- kernel A inputs
    xTb = [_bf(x[b].T) for b in range(B)]
    in_maps = []
    for b, q in core_bq:
        sl = slice(q * CH, (q + 1) * CH)
        w1 = np.concatenate([in_proj_w[sl], in_proj_w[D_INNER + q * CH:
                                                      D_INNER + (q + 1) * CH]], 0)
        cw = conv_w[sl, 0, :]                            # [CH, 4]
        cd = np.zeros((D_CONV * NCB * 128, 128), np.float32)
        for tap in range(D_CONV):
            for cb in range(NCB):
                blk = cd[(tap * NCB + cb) * 128:(tap * NCB + cb + 1) * 128]
                np.fill_diagonal(blk, cw[cb * 128:(cb + 1) * 128, tap])
        in_maps.append({
            "xT": xTb[b],
            "w1t": _bf(w1.T),
            "convdiag": _bf(cd),
            "convb": _f32(conv_b[sl].reshape(NCB, 128).T),
            "wxpT": _bf(x_proj_w[:, sl].T),
        })
    ra = run_bass_kernel_spmd(nca, in_maps, list(range(NCORES)))

    # ---------------- host exchange
    dbc = [None, None]
    for b in range(B):
        dbc[b] = sum(np.asarray(ra.results[4 * b + q]["dbc"], np.float32)
                     for q in range(4))
    in_maps_b = []
    breps, creps = [], []
    for b in range(B):
        Bm = dbc[b][DT_RANK:DT_RANK + D_STATE]           # [16, L]
        Cm = dbc[b][DT_RANK + D_STATE:]
        breps.append(_bf(np.repeat(Bm, 128, axis=0)))
        creps.append(_bf(np.repeat(Cm, 128, axis=0)))
    for c, (b, q) in enumerate(core_bq):
        sl = slice(q * CH, (q + 1) * CH)
        acol = np.zeros((128, D_STATE * NCB), np.float32)
        for s in range(D_STATE):
            for cb in range(NCB):
                acol[:, s * NCB + cb] = A[q * CH + cb * 128:
                                          q * CH + (cb + 1) * 128, s]
        in_maps_b.append({
            "xc": ra.results[c]["xc"],
            "sres": ra.results[c]["sres"],
            "dtr": _f32(dbc[b][:DT_RANK]),
            "brep": breps[b],
            "crep": creps[b],
            "wdtT": _f32(dt_proj_w[sl].T),
            "dtb": _f32(dt_proj_b[sl].reshape(NCB, 128).T),
            "woutT": _bf(out_proj_w[:, sl].T),
            "acol": acol,
            "dcol": _f32(D[sl].reshape(NCB, 128).T),
            "ident": _bf(np.eye(128, dtype=np.float32)),
        })
    rb = run_bass_kernel_spmd(ncb, in_maps_b, list(range(NCORES)))

    out = np.zeros((B, L, D_MODEL), np.float32)
    for b in range(B):
        acc = sum(np.asarray(rb.results[4 * b + q]["outp"], np.float32)
                  for q in range(4))
        out[b] = acc.T
    return out
